# revision 1
# baseline (speedup 1.0000x reference)
"""Trainium2 Bass kernel for: out = SCALE * x @ weight.sum(axis=0).

Strategy (8 cores, data-parallel):
  - x [16384, 4096] f32 row-sharded -> 8 x [2048, 4096]
  - weight [4096, 4096] f32 row-sharded -> 8 x [512, 4096]
  - per core: partial wsum = colsum(w_shard) via DVE adds + PE ones-matmul,
    16KB AllReduce over 8 cores, partition_broadcast to [128, 4096],
    then stream x tiles through one fused DVE tensor_tensor_reduce each:
      out_tile = (x_tile * wsumB) * SCALE ; accum = rowsum(out_tile)
  - per-core output [128, n_xt] is transposed/flattened on host.
"""

import numpy as np

from concourse import bacc, bass, tile
import concourse.mybir as mybir
from concourse.bass_utils import run_bass_kernel_spmd

N_CORES = 8
BATCH = 16384
IN_SIZE = 4096
W_ROWS = 4096
SCALE = 0.5
P = 128
MM_N = 512  # one fp32 PSUM bank


def build_nc(
    batch_shard: int,
    in_size: int,
    w_rows_shard: int,
    n_cores: int,
    scale: float,
    stream_bufs: int = 9,
    for_sim: bool = False,
    reps: int = 1,
    dma_split: bool = False,
):
    """Build the per-core SPMD Bass program."""
    fp32 = mybir.dt.float32
    n_wt = w_rows_shard // P
    n_xt = batch_shard // P
    assert w_rows_shard % P == 0 and batch_shard % P == 0
    assert in_size % MM_N == 0

    if for_sim:
        nc = bacc.Bacc(
            None, target_bir_lowering=False, debug=True, num_devices=n_cores
        )
    else:
        nc = bacc.Bacc(None, num_devices=n_cores)
    x_ext = nc.declare_dram_parameter(
        "x_shard", [batch_shard, in_size], fp32, isOutput=False
    )
    w_ext = nc.declare_dram_parameter(
        "w_shard", [w_rows_shard, in_size], fp32, isOutput=False
    )
    out_ext = nc.declare_dram_parameter("out_shard", [P, n_xt], fp32, isOutput=True)

    with tile.TileContext(nc) as tc:
        with (
            tc.tile_pool(name="stream", bufs=stream_bufs) as stream,
            tc.tile_pool(name="aux", bufs=1) as aux,
            tc.tile_pool(name="psum", bufs=1, space="PSUM") as psum,
            tc.tile_pool(name="dram", bufs=1, space="DRAM") as dram,
        ):
            # --- weight shard -> partial column sum ---------------------
            wt = []
            for k in range(n_wt):
                t = stream.tile([P, in_size], fp32, tag="stream")
                nc.sync.dma_start(out=t[:], in_=w_ext[k * P : (k + 1) * P, :])
                wt.append(t)
            # pairwise tree-add onto wt[0] (DVE), releasing other slots
            stride = 1
            while stride < n_wt:
                for k in range(0, n_wt, 2 * stride):
                    if k + stride < n_wt:
                        nc.vector.tensor_add(wt[k][:], wt[k][:], wt[k + stride][:])
                stride *= 2
            wacc = wt[0]

            ones = aux.tile([P, 1], fp32)
            nc.vector.memset(ones[:], 1.0)
            pw = psum.tile([1, in_size], fp32)
            for j in range(in_size // MM_N):
                nc.tensor.matmul(
                    pw[0:1, j * MM_N : (j + 1) * MM_N],
                    ones[:],
                    wacc[:, j * MM_N : (j + 1) * MM_N],
                    start=True,
                    stop=True,
                )

            # --- AllReduce the 16KB partial wsum across cores -----------
            wrow = aux.tile([1, in_size], fp32)
            nc.scalar.copy(wrow[:], pw[0:1, :])
            cc_in = dram.tile([1, in_size], fp32)
            cc_out = dram.tile([1, in_size], fp32)
            nc.gpsimd.dma_start(out=cc_in[:], in_=wrow[:])
            nc.gpsimd.collective_compute(
                "AllReduce",
                mybir.AluOpType.add,
                replica_groups=[list(range(n_cores))],
                ins=[cc_in[:].opt()],
                outs=[cc_out[:].opt()],
            )

            # --- broadcast wsum to all 128 partitions -------------------
            nc.gpsimd.dma_start(out=wrow[:], in_=cc_out[:])
            wsumB = aux.tile([P, in_size], fp32)
            nc.gpsimd.partition_broadcast(wsumB[:], wrow[:])

            # --- stream x tiles: fused mul + scale + row-sum ------------
            osb = aux.tile([P, n_xt], fp32)
            for rep in range(reps):
                for t in range(n_xt):
                    xt = stream.tile([P, in_size], fp32, tag="stream")
                    eng = nc.scalar if (dma_split and t % 2) else nc.sync
                    eng.dma_start(out=xt[:], in_=x_ext[t * P : (t + 1) * P, :])
                    nc.vector.scalar_tensor_tensor(
                        out=xt[:],
                        in0=xt[:],
                        scalar=scale,
                        in1=wsumB[:],
                        op0=mybir.AluOpType.mult,
                        op1=mybir.AluOpType.mult,
                        accum_out=osb[:, t : t + 1],
                    )

            nc.gpsimd.dma_start(out=out_ext[:], in_=osb[:])

    return nc


def build_nc_cols(
    batch: int,
    cols: int,
    w_rows: int,
    scale: float,
    stream_bufs: int = 7,
    for_sim: bool = False,
    reps: int = 1,
    dma_split: bool = True,
    w_split: bool = True,
    pool_every: int = 0,
    act_offload: bool = False,
    w_g: int = 2,
    act_every: int = 0,
):
    """Column-sharded variant: per core, x_cols [batch, cols] and
    w_cols [w_rows, cols]; output osb [128, batch//128] of partial dot
    products (host sums across cores). No collective.

    Tiles are [128, G, cols] covering G*128 batch (or weight) rows.
    pool_every=N>0 sends every Nth reduce chunk to GPSIMD."""
    fp32 = mybir.dt.float32
    G = 8
    rows_per_tile = P * G
    n_wt = w_rows // rows_per_tile
    n_xt = batch // rows_per_tile
    assert batch % rows_per_tile == 0 and w_rows % rows_per_tile == 0
    assert cols <= MM_N  # one PSUM bank / matmul chunk

    if for_sim:
        nc = bacc.Bacc(None, target_bir_lowering=False, debug=True, num_devices=1)
    else:
        nc = bacc.Bacc(None, num_devices=N_CORES)
    x_ext = nc.declare_dram_parameter("x_cols", [batch, cols], fp32, isOutput=False)
    w_ext = nc.declare_dram_parameter("w_cols", [w_rows, cols], fp32, isOutput=False)
    out_ext = nc.declare_dram_parameter(
        "out_part", [P, batch // P], fp32, isOutput=True
    )

    with tile.TileContext(nc) as tc:
        with (
            tc.tile_pool(name="stream", bufs=stream_bufs) as stream,
            tc.tile_pool(name="wpool", bufs=16) as wpool,
            tc.tile_pool(name="aux", bufs=1) as aux,
            tc.tile_pool(name="psum", bufs=1, space="PSUM") as psum,
        ):
            # --- weight stripe -> local wsum[cols] ----------------------
            # smaller tiles [P, Gw, cols] so the add tree pipelines with
            # DMA arrivals instead of waiting for 2MB tiles
            Gw = w_g
            w_rows_per_tile = P * Gw
            n_wt8 = w_rows // w_rows_per_tile
            wt = []
            for k in range(n_wt8):
                t = wpool.tile([P, Gw, cols], fp32, tag="wtile")
                src = w_ext[k * w_rows_per_tile : (k + 1) * w_rows_per_tile, :]
                eng = nc.scalar if (w_split and k % 2) else nc.sync
                eng.dma_start(out=t[:], in_=src.rearrange("(g p) f -> p g f", p=P))
                wt.append(t)
            stride = 1
            while stride < n_wt8:
                for k in range(0, n_wt8, 2 * stride):
                    if k + stride < n_wt8:
                        nc.vector.tensor_add(wt[k][:], wt[k][:], wt[k + stride][:])
                stride *= 2
            # fold the Gw blocks down to one [P, cols] block
            g = Gw
            while g > 1:
                g //= 2
                nc.vector.tensor_add(
                    wt[0][:, 0:g, :], wt[0][:, 0:g, :], wt[0][:, g : 2 * g, :]
                )
            ones = aux.tile([P, 1], fp32)
            nc.vector.memset(ones[:], 1.0)
            pw = psum.tile([1, cols], fp32)
            nc.tensor.matmul(pw[0:1, :], ones[:], wt[0][:, 0, :], start=True, stop=True)
            wrow = aux.tile([1, cols], fp32)
            nc.vector.tensor_copy(wrow[:], pw[0:1, :])
            wsumB = aux.tile([P, cols], fp32)
            nc.gpsimd.partition_broadcast(wsumB[:], wrow[:])

            # --- stream x tiles ----------------------------------------
            osb = aux.tile([P, batch // P], fp32)
            for rep in range(reps):
                for t in range(n_xt):
                    xt = stream.tile([P, G, cols], fp32, tag="stream")
                    src = x_ext[t * rows_per_tile : (t + 1) * rows_per_tile, :]
                    eng = nc.scalar if (dma_split and t % 2) else nc.sync
                    eng.dma_start(
                        out=xt[:], in_=src.rearrange("(g p) f -> p g f", p=P)
                    )
                    act_tile = act_every > 0 and t % act_every == 0
                    if act_tile and not act_offload:
                        # DVE big multiply, ACT per-chunk accumulate
                        nc.vector.tensor_tensor(
                            out=xt[:],
                            in0=xt[:],
                            in1=wsumB[:, None, :].broadcast_to([P, G, cols]),
                            op=mybir.AluOpType.mult,
                        )
                        for gi in range(G):
                            col = t * G + gi
                            nc.scalar.activation(
                                out=xt[:, gi, :],
                                in_=xt[:, gi, :],
                                func=mybir.ActivationFunctionType.Copy,
                                scale=scale,
                                accum_out=osb[:, col : col + 1],
                            )
                    elif act_offload:
                        # one big DVE multiply, then per-chunk ACT accumulate
                        nc.vector.tensor_tensor(
                            out=xt[:],
                            in0=xt[:],
                            in1=wsumB[:, None, :].broadcast_to([P, G, cols]),
                            op=mybir.AluOpType.mult,
                        )
                        for gi in range(G):
                            col = t * G + gi
                            nc.scalar.activation(
                                out=xt[:, gi, :],
                                in_=xt[:, gi, :],
                                func=mybir.ActivationFunctionType.Copy,
                                scale=scale,
                                accum_out=osb[:, col : col + 1],
                            )
                    else:
                        for gi in range(G):
                            col = t * G + gi
                            use_pool = pool_every > 0 and (
                                col % pool_every == pool_every - 1
                            )
                            veng = nc.gpsimd if use_pool else nc.vector
                            veng.scalar_tensor_tensor(
                                out=xt[:, gi, :],
                                in0=xt[:, gi, :],
                                scalar=scale,
                                in1=wsumB[:],
                                op0=mybir.AluOpType.mult,
                                op1=mybir.AluOpType.mult,
                                accum_out=osb[:, col : col + 1],
                            )

            nc.sync.dma_start(out=out_ext[:], in_=osb[:])

    return nc


_NC_CACHE: dict = {}


def _get_nc():
    key = ("cols", BATCH, IN_SIZE // N_CORES, W_ROWS)
    if key not in _NC_CACHE:
        nc = build_nc_cols(BATCH, IN_SIZE // N_CORES, W_ROWS, SCALE)
        nc.finalize()
        _NC_CACHE[key] = nc
    return _NC_CACHE[key]


def _make_in_maps(x: np.ndarray, weight: np.ndarray):
    cs = IN_SIZE // N_CORES
    return [
        {
            "x_cols": np.ascontiguousarray(x[:, c * cs : (c + 1) * cs]),
            "w_cols": np.ascontiguousarray(weight[:, c * cs : (c + 1) * cs]),
        }
        for c in range(N_CORES)
    ]


def _assemble(results) -> np.ndarray:
    # per-core out_part is [P, batch//P] with [p, t] = partial[t*P + p]
    acc = None
    for c in range(N_CORES):
        o = np.asarray(results[c]["out_part"]).T.reshape(-1)
        acc = o if acc is None else acc + o
    return acc.astype(np.float32)


def kernel(x: np.ndarray, weight: np.ndarray) -> np.ndarray:
    x = np.asarray(x, dtype=np.float32)
    weight = np.asarray(weight, dtype=np.float32)
    assert x.shape == (BATCH, IN_SIZE) and weight.shape == (W_ROWS, IN_SIZE)
    nc = _get_nc()
    res = run_bass_kernel_spmd(
        nc, _make_in_maps(x, weight), list(range(N_CORES))
    ).results
    return _assemble(res)



# revision 9
# speedup vs baseline: 1.6657x; 1.6657x over previous
"""Trainium2 Bass kernel for: out = SCALE * x @ weight.sum(axis=0).

Column-sharded over 8 cores (stripe of 512 cols each); every core computes
partial dot products for ALL 16384 batch rows over its stripe; host sums the
8 partials.

Per-core pipeline (all device inputs bf16, host casts during sharding):
  Phase W (wsum = colsum of the weight stripe):
    - w_t  [512, W_ACT+W_DVE] (transposed rows-share): ACT accum + DVE reduce
      -> column-form partials [128, 4]
    - w_nat [W_PE, 512] (natural rows-share): PE ones-matmul -> row-form
      partial [1, 512] in PSUM
    - fold: col partials -> row via SBUF-SBUF DMA, add, partition_broadcast
      (row form for DVE), and row -> col DMA + bf16 copy (lhsT form for PE)
  Phase X (dots):
    - x_pe_t [512, B_PE] (transposed): PE matvec, 4 chunk-matmuls per
      512-batch group accumulated in a PSUM bank slot; ACT evicts slots with
      SCALE into SBUF
    - x_dve [B_DVE, 512] (natural): DVE fused scalar_tensor_tensor with
      accum_out per 128-row block
  DMA: three parallel streams - sync HWDGE, scalar HWDGE, gpsimd SWDGE
  gathers (iota index tile makes gathers generic consecutive-row loads).
"""

import numpy as np
import ml_dtypes

from concourse import bacc, bass, tile
import concourse.mybir as mybir
from concourse.bass_utils import run_bass_kernel_spmd

BF16NP = ml_dtypes.bfloat16

N_CORES = 8
BATCH = 16384
IN_SIZE = 4096
CS = IN_SIZE // N_CORES  # 512 cols per core
W_ROWS = 4096
SCALE = 0.5
P = 128

# --- tunables ----------------------------------------------------------------
B_PE = 12288            # batch rows handled by PE (24 groups of 512)
B_DVE = BATCH - B_PE    # batch rows handled by DVE (32 blocks of 128)
W_PE = 2560             # weight rows summed by PE (natural layout)
W_ACT = 768             # weight rows summed by ACT (transposed layout)
W_DVE = W_ROWS - W_PE - W_ACT  # 768, weight rows summed by DVE (transposed)
XPE_WIN = 2048          # batch window per x_pe tile ([128, 4, XPE_WIN] bf16)
N_WIN = B_PE // XPE_WIN  # 6 windows
# per-window queue: 'g' = gpsimd gather, 's' = scalar HWDGE
XPE_Q = "ggggss"
# w_nat 128-row blocks split between queues: gather / sync / scalar
WNAT_GATHER = 12        # blocks via gather (1536 rows)
WNAT_SYNC = 4
WNAT_SCALAR = 4
N_GROUPS = B_PE // 512  # 24 PSUM accumulation groups
N_SLOTS = (N_GROUPS + 7) // 8  # 3 psum partition-slots (0,32,64)
N_DBLK = B_DVE // P     # 32 DVE blocks

bf16 = mybir.dt.bfloat16
fp32 = mybir.dt.float32
i16 = mybir.dt.int16


def build_nc(for_sim: bool = False):
    if for_sim:
        nc = bacc.Bacc(None, target_bir_lowering=False, debug=True, num_devices=1)
    else:
        nc = bacc.Bacc(None, num_devices=N_CORES)

    x_pe_t = nc.declare_dram_parameter("x_pe_t", [CS, B_PE], bf16, isOutput=False)
    x_dve = nc.declare_dram_parameter("x_dve", [B_DVE, CS], bf16, isOutput=False)
    w_nat = nc.declare_dram_parameter("w_nat", [W_PE, CS], bf16, isOutput=False)
    w_t = nc.declare_dram_parameter("w_t", [CS, W_ACT + W_DVE], bf16, isOutput=False)
    idx_ext = nc.declare_dram_parameter("idx", [16, 1024], i16, isOutput=False)
    ident_ext = nc.declare_dram_parameter("ident", [P, P], fp32, isOutput=False)
    out_pe = nc.declare_dram_parameter("out_pe", [N_SLOTS, 4096], fp32, isOutput=True)
    out_dve = nc.declare_dram_parameter("out_dve", [P, N_DBLK], fp32, isOutput=True)

    with tile.TileContext(nc) as tc:
        with (
            tc.tile_pool(name="xpe", bufs=3) as xpe_pool,
            tc.tile_pool(name="xdve", bufs=2) as xdve_pool,
            tc.tile_pool(name="wpool", bufs=1) as wpool,
            tc.tile_pool(name="aux", bufs=1) as aux,
            tc.tile_pool(name="psum", bufs=1, space="PSUM") as psum,
        ):
            # --- index tile for gathers (iota: idx[p, s] = s*16 + p) --------
            idxt = aux.tile([P, 1024], i16)
            nc.vector.memset(idxt[:], 0)
            nc.sync.dma_start(out=idxt[0:16, :], in_=idx_ext[:, :])
            ident = aux.tile([P, P], fp32)
            nc.scalar.dma_start(out=ident[:], in_=ident_ext[:, :])

            # =================== Phase W: wsum ==============================
            # transposed shares first (they unblock the fold earliest)
            wtA = []
            wtD = []
            for c in range(4):
                ta = wpool.tile([P, W_ACT], bf16, tag=f"wtA{c}")
                eng = nc.sync if c < 2 else nc.scalar
                eng.dma_start(out=ta[:], in_=w_t[c * P:(c + 1) * P, 0:W_ACT])
                wtA.append(ta)
            for c in range(4):
                td = wpool.tile([P, W_DVE], bf16, tag=f"wtD{c}")
                eng = nc.sync if c < 2 else nc.scalar
                eng.dma_start(out=td[:], in_=w_t[c * P:(c + 1) * P, W_ACT:])
                wtD.append(td)

            # natural share (PE): split across queues
            n_wb = W_PE // P  # 20 blocks
            wnat_g = wpool.tile([P, WNAT_GATHER, CS], bf16, tag="wnat_g")
            nc.gpsimd.dma_gather(
                wnat_g[:], w_nat[:, :], idxt[:, 0:(WNAT_GATHER * P) // 16],
                WNAT_GATHER * P, WNAT_GATHER * P, CS, elem_step=CS, queue_num=0,
            )
            wnat_s1 = wpool.tile([P, WNAT_SYNC, CS], bf16, tag="wnat_s1")
            r0 = WNAT_GATHER * P
            nc.sync.dma_start(
                out=wnat_s1[:],
                in_=w_nat[r0:r0 + WNAT_SYNC * P, :].rearrange("(g p) f -> p g f", p=P),
            )
            wnat_s2 = wpool.tile([P, WNAT_SCALAR, CS], bf16, tag="wnat_s2")
            r1 = r0 + WNAT_SYNC * P
            nc.scalar.dma_start(
                out=wnat_s2[:],
                in_=w_nat[r1:r1 + WNAT_SCALAR * P, :].rearrange("(g p) f -> p g f", p=P),
            )

            # ACT: accumulate transposed tiles -> col partials
            wcol = aux.tile([P, 4], fp32)
            for c in range(4):
                nc.scalar.activation(
                    out=wtA[c][:], in_=wtA[c][:],
                    func=mybir.ActivationFunctionType.Copy,
                    accum_out=wcol[:, c:c + 1],
                )
            # DVE: reduce transposed tiles -> col partials
            wcolD = aux.tile([P, 4], fp32)
            for c in range(4):
                nc.vector.tensor_reduce(
                    out=wcolD[:, c:c + 1], in_=wtD[c][:],
                    axis=mybir.AxisListType.X, op=mybir.AluOpType.add,
                )
            nc.vector.tensor_tensor(
                out=wcol[:], in0=wcol[:], in1=wcolD[:], op=mybir.AluOpType.add
            )

            # PE: ones-matmul over natural blocks -> row partial in PSUM
            ones = aux.tile([P, 1], bf16)
            nc.vector.memset(ones[:], 1.0)
            pw = psum.tile([P, 4096], fp32)
            mm_idx = 0
            for tile_, nblk in ((wnat_g, WNAT_GATHER), (wnat_s1, WNAT_SYNC),
                                (wnat_s2, WNAT_SCALAR)):
                for g in range(nblk):
                    nc.tensor.matmul(
                        pw[0:1, 0:CS], ones[:], tile_[:, g, :],
                        start=(mm_idx == 0), stop=(mm_idx == n_wb - 1),
                    )
                    mm_idx += 1

            # fold: col partials -> row form via PE transposes into psum bank1
            for c in range(4):
                nc.tensor.matmul(
                    pw[0:1, 512 + c * P: 512 + (c + 1) * P],
                    wcol[:, c:c + 1], ident[:, :],
                    is_transpose=True, start=True, stop=True,
                )
            wrow = aux.tile([1, CS], fp32)
            nc.vector.tensor_tensor(
                out=wrow[:], in0=pw[0:1, 0:CS], in1=pw[0:1, 512:1024],
                op=mybir.AluOpType.add,
            )
            # row -> broadcast (DVE x-share) and col lhsT form (PE x-share)
            wsumB = aux.tile([P, CS], fp32)
            nc.gpsimd.partition_broadcast(wsumB[:], wrow[:])
            for c in range(4):
                nc.tensor.matmul(
                    pw[0:P, 1024 + c: 1024 + c + 1],
                    wrow[0:1, c * P:(c + 1) * P], ident[0:1, 0:1],
                    is_transpose=True, start=True, stop=True,
                )
            wcolT_bf = aux.tile([P, 4], bf16)
            nc.vector.tensor_copy(wcolT_bf[:], pw[0:P, 1024:1028])

            # =================== Phase X: dot products ======================
            osb_pe = aux.tile([P, 4096], fp32)
            osb_dve = aux.tile([P, N_DBLK], fp32)

            # interleave issue: PE windows and DVE tiles
            n_dve_tiles = N_DBLK // 8  # [128, 8, 512] tiles
            dve_done = 0
            for w in range(N_WIN):
                xt = xpe_pool.tile([P, 4, XPE_WIN], bf16, tag="xpe")
                if XPE_Q[w] == "g":
                    nc.gpsimd.dma_gather(
                        xt[:], x_pe_t[:, w * XPE_WIN:(w + 1) * XPE_WIN],
                        idxt[:, 0:CS // 16], CS, CS, XPE_WIN,
                        elem_step=B_PE, queue_num=0,
                    )
                else:
                    nc.scalar.dma_start(
                        out=xt[:],
                        in_=x_pe_t[:, w * XPE_WIN:(w + 1) * XPE_WIN].rearrange(
                            "(c p) f -> p c f", p=P),
                    )
                for sub in range(XPE_WIN // 512):
                    g = w * (XPE_WIN // 512) + sub
                    bank, slot = g % 8, g // 8
                    ps = pw[slot * 32: slot * 32 + 1, bank * 512:(bank + 1) * 512]
                    for c in range(4):
                        nc.tensor.matmul(
                            ps, wcolT_bf[:, c:c + 1],
                            xt[:, c, sub * 512:(sub + 1) * 512],
                            start=(c == 0), stop=(c == 3),
                        )
                    if g % 8 == 7:
                        slot_done = g // 8
                        nc.scalar.activation(
                            out=osb_pe[slot_done * 32: slot_done * 32 + 1, :],
                            in_=pw[slot_done * 32: slot_done * 32 + 1, :],
                            func=mybir.ActivationFunctionType.Copy,
                            scale=SCALE,
                        )
                # interleave one DVE tile per PE window
                if dve_done < n_dve_tiles:
                    t = dve_done
                    dve_done += 1
                    xd = xdve_pool.tile([P, 8, 512], bf16, tag="xdve")
                    nc.sync.dma_start(
                        out=xd[:],
                        in_=x_dve[t * 8 * P:(t + 1) * 8 * P, :].rearrange(
                            "(g p) f -> p g f", p=P),
                    )
                    for gi in range(8):
                        col = t * 8 + gi
                        nc.vector.scalar_tensor_tensor(
                            out=xd[:, gi, :], in0=xd[:, gi, :], scalar=SCALE,
                            in1=wsumB[:],
                            op0=mybir.AluOpType.mult, op1=mybir.AluOpType.mult,
                            accum_out=osb_dve[:, col:col + 1],
                        )

            for s in range(N_SLOTS):
                nc.gpsimd.dma_start(
                    out=out_pe[s:s + 1, :], in_=osb_pe[s * 32:s * 32 + 1, :]
                )
            nc.gpsimd.dma_start(out=out_dve[:], in_=osb_dve[:])

    return nc


_NC_CACHE: dict = {}


def _get_nc():
    if "nc" not in _NC_CACHE:
        nc = build_nc()
        nc.finalize()
        _NC_CACHE["nc"] = nc
    return _NC_CACHE["nc"]


_IDX = None


def _iota_idx() -> np.ndarray:
    global _IDX
    if _IDX is None:
        v = np.arange(16384, dtype=np.int16)
        _IDX = np.ascontiguousarray(v.reshape(1024, 16).T)  # idx[p, s] = s*16+p
    return _IDX


def make_in_maps(x: np.ndarray, weight: np.ndarray):
    idx = _iota_idx()
    maps = []
    for c in range(N_CORES):
        sl = slice(c * CS, (c + 1) * CS)
        xs = x[:, sl].astype(BF16NP)
        ws = weight[:, sl].astype(BF16NP)
        maps.append({
            "x_pe_t": np.ascontiguousarray(xs[:B_PE].T),
            "x_dve": np.ascontiguousarray(xs[B_PE:]),
            "w_nat": np.ascontiguousarray(ws[:W_PE]),
            "w_t": np.ascontiguousarray(ws[W_PE:].T),
            "idx": idx,
            "ident": np.eye(P, dtype=np.float32),
        })
    return maps


def assemble(results) -> np.ndarray:
    out = np.zeros(BATCH, dtype=np.float64)
    for c in range(N_CORES):
        ope = np.asarray(results[c]["out_pe"], dtype=np.float64)
        odv = np.asarray(results[c]["out_dve"], dtype=np.float64)
        # out_pe[slot, bank*512+j] -> batch (slot*8+bank)*512 + j
        out[:B_PE] += ope.reshape(-1)[:B_PE]
        # out_dve[p, t] -> batch B_PE + t*128 + p
        out[B_PE:] += odv.T.reshape(-1)
    return out.astype(np.float32)


def kernel(x: np.ndarray, weight: np.ndarray) -> np.ndarray:
    x = np.asarray(x, dtype=np.float32)
    weight = np.asarray(weight, dtype=np.float32)
    assert x.shape == (BATCH, IN_SIZE) and weight.shape == (W_ROWS, IN_SIZE)
    nc = _get_nc()
    res = run_bass_kernel_spmd(nc, make_in_maps(x, weight), list(range(N_CORES))).results
    return assemble(res)


# revision 12
# speedup vs baseline: 2.1751x; 1.3058x over previous
"""Trainium2 Bass kernel for: out = SCALE * x @ weight.sum(axis=0).

Column-sharded over 8 cores (stripe of 512 cols each); every core computes
partial dot products for ALL 16384 batch rows over its stripe; host sums the
8 partials. All device inputs bf16 (host casts during sharding; tolerance is
2e-2 so bf16 inputs with fp32 accumulation are well within budget).

Per-core pipeline:
  Phase W (wsum = colsum of the weight stripe):
    - w_t [512, W_ACT+W_DVE] transposed rows-share: ACT accum_out + DVE
      tensor_reduce -> column-form partials [128, 4] (f32)
    - w_nat [W_PE, 512] natural rows-share: PE ones-matmul -> row partial
      [1, 512] in PSUM
    - fold (PE transposes with an identity tile, no DMA): col->row, add,
      then row->all-partitions broadcast via ones-matmul (wsumB), and
      row->col lhsT form (wcolT, bf16)
  Phase X:
    - PE share: x gathered/DMA'd in transposed layout; 4 chunk-matmuls per
      512-batch group accumulate into a PSUM bank slot (partitions 0/32/64);
      ACT evicts finished slots with SCALE
    - DVE share: natural layout, fused scalar_tensor_tensor per 128-row
      block with accum_out
  DMA streams (run in parallel, each serialized on its queue):
    sync HWDGE ~350 B/ns, scalar HWDGE ~350 B/ns, gpsimd SWDGE gathers
    ~600 B/ns (gathers shaped num_idxs=1024 x 2KB rows, which pipeline
    back-to-back; x for the PE share is host-packed as two stacked
    batch-halves so each gather covers 2048 batch rows).
"""

import numpy as np
import ml_dtypes

from concourse import bacc, bass, tile
import concourse.mybir as mybir
from concourse.bass_utils import run_bass_kernel_spmd

BF16NP = ml_dtypes.bfloat16

N_CORES = 8
BATCH = 16384
IN_SIZE = 4096
CS = IN_SIZE // N_CORES  # 512 cols per core
W_ROWS = 4096
SCALE = 0.5
P = 128

# --- tunables ---------------------------------------------------------------
B_PEG = 10240           # PE batch via gathers (5 windows of 2048)
B2 = B_PEG // 2         # stacked-half width (5120)
B_PES = 1536            # PE batch via scalar HWDGE (3 groups)
B_PE = B_PEG + B_PES    # 11776
B_DVE = BATCH - B_PE    # 4608 -> 36 DVE blocks
W_PE = 1792             # weight rows summed by PE (natural), 14 blocks
W_ACT = 1280            # weight rows summed by ACT (transposed)
W_DVE = W_ROWS - W_PE - W_ACT  # 1024, summed by DVE (transposed)
N_GWIN = B_PEG // 2048  # 5 gather windows
N_DBLK = B_DVE // P     # 36 DVE blocks
N_DTILE = N_DBLK // 4   # 9 tiles of [128, 4, 512]
N_GROUPS = B_PE // 512  # 23 PSUM groups

bf16 = mybir.dt.bfloat16
fp32 = mybir.dt.float32
i16 = mybir.dt.int16


def _emit_order():
    """PSUM group emission order: (kind, window, sub) tuples.
    kind 'g': gather window w (4 groups: h in 0,1 x k in 0,1),
    kind 's': scalar window (3 groups)."""
    order = []
    order += [("g", 0, s) for s in range(4)]
    order += [("s", 0, s) for s in range(3)]
    for w in range(1, N_GWIN):
        order += [("g", w, s) for s in range(4)]
    return order


EMIT = _emit_order()


def batch_base(kind, w, s):
    if kind == "g":
        h, k = s // 2, s % 2
        return h * B2 + w * 1024 + k * 512
    return B_PEG + s * 512


def build_nc(for_sim: bool = False):
    if for_sim:
        nc = bacc.Bacc(None, target_bir_lowering=False, debug=True, num_devices=1)
    else:
        nc = bacc.Bacc(None, num_devices=N_CORES)

    x_pe_g = nc.declare_dram_parameter("x_pe_g", [1024, B2], bf16, isOutput=False)
    x_pe_s = nc.declare_dram_parameter("x_pe_s", [CS, B_PES], bf16, isOutput=False)
    x_dve = nc.declare_dram_parameter("x_dve", [B_DVE, CS], bf16, isOutput=False)
    w_nat = nc.declare_dram_parameter("w_nat", [W_PE, CS], bf16, isOutput=False)
    w_t = nc.declare_dram_parameter("w_t", [CS, W_ACT + W_DVE], bf16, isOutput=False)
    idx_ext = nc.declare_dram_parameter("idx", [P, P], i16, isOutput=False)
    ident_ext = nc.declare_dram_parameter("ident", [P, P], fp32, isOutput=False)
    out_pe = nc.declare_dram_parameter("out_pe", [3, 4096], fp32, isOutput=True)
    out_dve = nc.declare_dram_parameter("out_dve", [P, N_DBLK], fp32, isOutput=True)

    with tile.TileContext(nc) as tc:
        with (
            tc.tile_pool(name="xg", bufs=5) as xg_pool,
            tc.tile_pool(name="xs", bufs=1) as xs_pool,
            tc.tile_pool(name="xd", bufs=9) as xd_pool,
            tc.tile_pool(name="wpool", bufs=1) as wpool,
            tc.tile_pool(name="aux", bufs=1) as aux,
            tc.tile_pool(name="psum", bufs=1, space="PSUM") as psum,
        ):
            # --- small constants (sync first: idx unblocks gathers) ---------
            idxt = aux.tile([P, P], i16)
            nc.sync.dma_start(out=idxt[:], in_=idx_ext[:, :])
            ident = aux.tile([P, P], fp32)
            nc.scalar.dma_start(out=ident[:], in_=ident_ext[:, :])

            # --- Pool queue: pure gather stream, back-to-back ---------------
            xg_tiles = []
            for w in range(N_GWIN):
                xt = xg_pool.tile([P, 8, 1024], bf16, tag="xg")
                nc.gpsimd.dma_gather(
                    xt[:], x_pe_g[:, w * 1024:(w + 1) * 1024],
                    idxt[:, 0:64], 1024, 1024, 1024,
                    elem_step=B2, queue_num=0,
                )
                xg_tiles.append(xt)

            # --- Phase W DMAs (sync+scalar): w_t first, then w_nat ----------
            wtA, wtD = [], []
            for c in range(4):
                ta = wpool.tile([P, W_ACT], bf16, tag=f"wtA{c}")
                (nc.sync if c < 2 else nc.scalar).dma_start(
                    out=ta[:], in_=w_t[c * P:(c + 1) * P, 0:W_ACT])
                wtA.append(ta)
            for c in range(4):
                td = wpool.tile([P, W_DVE], bf16, tag=f"wtD{c}")
                (nc.sync if c < 2 else nc.scalar).dma_start(
                    out=td[:], in_=w_t[c * P:(c + 1) * P, W_ACT:])
                wtD.append(td)
            nb_half = W_PE // P // 2  # 7 blocks per queue
            wnat_s1 = wpool.tile([P, nb_half, CS], bf16, tag="wnat_s1")
            nc.sync.dma_start(
                out=wnat_s1[:],
                in_=w_nat[0:nb_half * P, :].rearrange("(g p) f -> p g f", p=P))
            wnat_s2 = wpool.tile([P, nb_half, CS], bf16, tag="wnat_s2")
            nc.scalar.dma_start(
                out=wnat_s2[:],
                in_=w_nat[nb_half * P:, :].rearrange("(g p) f -> p g f", p=P))

            # --- Phase W compute -------------------------------------------
            wcol = aux.tile([P, 4], fp32)
            for c in range(4):
                nc.scalar.activation(
                    out=wtA[c][:], in_=wtA[c][:],
                    func=mybir.ActivationFunctionType.Copy,
                    accum_out=wcol[:, c:c + 1])
            wcolD = aux.tile([P, 4], fp32)
            for c in range(4):
                nc.vector.tensor_reduce(
                    out=wcolD[:, c:c + 1], in_=wtD[c][:],
                    axis=mybir.AxisListType.X, op=mybir.AluOpType.add)
            nc.vector.tensor_tensor(
                out=wcol[:], in0=wcol[:], in1=wcolD[:], op=mybir.AluOpType.add)

            ones = aux.tile([P, 1], bf16)
            nc.vector.memset(ones[:], 1.0)
            onesr = aux.tile([1, P], fp32)
            nc.vector.memset(onesr[:], 1.0)

            pw = psum.tile([P, 4096], fp32)
            n_wb = W_PE // P
            for i in range(n_wb):  # alternate queues to match arrival
                t_, g_ = (wnat_s1, i // 2) if i % 2 == 0 else (wnat_s2, i // 2)
                nc.tensor.matmul(
                    pw[0:1, 0:CS], ones[:], t_[:, g_, :],
                    start=(i == 0), stop=(i == n_wb - 1))

            # fold: col partials -> row (PE transposes), add, then both forms
            for c in range(4):
                nc.tensor.matmul(
                    pw[0:1, 512 + c * P: 512 + (c + 1) * P],
                    wcol[:, c:c + 1], ident[:, :],
                    is_transpose=True, start=True, stop=True)
            wrow = aux.tile([1, CS], fp32)
            nc.vector.tensor_tensor(
                out=wrow[:], in0=pw[0:1, 0:CS], in1=pw[0:1, 512:1024],
                op=mybir.AluOpType.add)
            # broadcast row to all partitions via ones-matmul (PSUM bank 3)
            nc.tensor.matmul(
                pw[0:P, 1536:2048], onesr[:], wrow[:],
                start=True, stop=True)
            wsumB = aux.tile([P, CS], fp32)
            nc.vector.tensor_copy(wsumB[:], pw[0:P, 1536:2048])
            # row -> col lhsT form
            for c in range(4):
                nc.tensor.matmul(
                    pw[0:P, 1024 + c: 1024 + c + 1],
                    wrow[0:1, c * P:(c + 1) * P], ident[0:1, 0:1],
                    is_transpose=True, start=True, stop=True)
            wcolT_bf = aux.tile([P, 4], bf16)
            nc.vector.tensor_copy(wcolT_bf[:], pw[0:P, 1024:1028])

            # --- Phase X ----------------------------------------------------
            # scalar window tile
            xst = xs_pool.tile([P, 4, B_PES], bf16, tag="xs")
            nc.scalar.dma_start(
                out=xst[:],
                in_=x_pe_s[:, :].rearrange("(c p) f -> p c f", p=P))

            osb_pe = aux.tile([P, 4096], fp32)
            osb_dve = aux.tile([P, N_DBLK], fp32)

            # sync queue: x_dve tiles
            xd_tiles = []
            for t in range(N_DTILE):
                xd = xd_pool.tile([P, 4, CS], bf16, tag="xd")
                nc.sync.dma_start(
                    out=xd[:],
                    in_=x_dve[t * 4 * P:(t + 1) * 4 * P, :].rearrange(
                        "(g p) f -> p g f", p=P))
                xd_tiles.append(xd)

            # PE groups in expected arrival order; DVE interleaved
            dve_t = 0

            def emit_dve_tile():
                nonlocal dve_t
                if dve_t >= N_DTILE:
                    return
                xd = xd_tiles[dve_t]
                for gi in range(4):
                    col = dve_t * 4 + gi
                    nc.vector.scalar_tensor_tensor(
                        out=xd[:, gi, :], in0=xd[:, gi, :], scalar=SCALE,
                        in1=wsumB[:],
                        op0=mybir.AluOpType.mult, op1=mybir.AluOpType.mult,
                        accum_out=osb_dve[:, col:col + 1])
                dve_t += 1

            for e, (kind, w, s) in enumerate(EMIT):
                bank, slot = e % 8, e // 8
                ps = pw[slot * 32: slot * 32 + 1, bank * 512:(bank + 1) * 512]
                if kind == "g":
                    xt = xg_tiles[w]
                    h, k = s // 2, s % 2
                    for c in range(4):
                        nc.tensor.matmul(
                            ps, wcolT_bf[:, c:c + 1],
                            xt[:, h * 4 + c, k * 512:(k + 1) * 512],
                            start=(c == 0), stop=(c == 3))
                else:
                    for c in range(4):
                        nc.tensor.matmul(
                            ps, wcolT_bf[:, c:c + 1],
                            xst[:, c, s * 512:(s + 1) * 512],
                            start=(c == 0), stop=(c == 3))
                if e == 15:
                    # slots 0+1 full: evict both in one ACT pass
                    nc.scalar.activation(
                        out=osb_pe[0:33:32, :], in_=pw[0:33:32, :],
                        func=mybir.ActivationFunctionType.Copy, scale=SCALE)
                elif e == 19:
                    nc.scalar.activation(
                        out=osb_pe[64:65, 0:2048], in_=pw[64:65, 0:2048],
                        func=mybir.ActivationFunctionType.Copy, scale=SCALE)
                elif e == 22:
                    nc.scalar.activation(
                        out=osb_pe[64:65, 2048:3584], in_=pw[64:65, 2048:3584],
                        func=mybir.ActivationFunctionType.Copy, scale=SCALE)
                # pace DVE: ~2 tiles of DVE work per 2 PE windows
                if e % 4 == 1:
                    emit_dve_tile()
                    emit_dve_tile()
            while dve_t < N_DTILE:
                emit_dve_tile()

            for s in range(3):
                ncols = 4096 if s < 2 else 3584
                nc.gpsimd.dma_start(
                    out=out_pe[s:s + 1, 0:ncols],
                    in_=osb_pe[s * 32:s * 32 + 1, 0:ncols])
            nc.gpsimd.dma_start(out=out_dve[:], in_=osb_dve[:])

    return nc


_NC_CACHE: dict = {}


def _get_nc():
    if "nc" not in _NC_CACHE:
        nc = build_nc()
        nc.finalize()
        _NC_CACHE["nc"] = nc
    return _NC_CACHE["nc"]


def _iota_idx() -> np.ndarray:
    idx = np.zeros((P, P), dtype=np.int16)
    for i in range(2048):
        idx[i % 16, i // 16] = i
    return idx


def make_in_maps(x: np.ndarray, weight: np.ndarray):
    idx = _iota_idx()
    ident = np.eye(P, dtype=np.float32)
    maps = []
    for c in range(N_CORES):
        sl = slice(c * CS, (c + 1) * CS)
        xs = x[:, sl].astype(BF16NP)
        ws = weight[:, sl].astype(BF16NP)
        xg = np.ascontiguousarray(xs[:B_PEG].T)          # [512, 10240]
        x_pe_g = np.ascontiguousarray(
            np.concatenate([xg[:, :B2], xg[:, B2:]], axis=0))  # [1024, B2]
        maps.append({
            "x_pe_g": x_pe_g,
            "x_pe_s": np.ascontiguousarray(xs[B_PEG:B_PE].T),
            "x_dve": np.ascontiguousarray(xs[B_PE:]),
            "w_nat": np.ascontiguousarray(ws[:W_PE]),
            "w_t": np.ascontiguousarray(ws[W_PE:].T),
            "idx": idx,
            "ident": ident,
        })
    return maps


def assemble(results) -> np.ndarray:
    out = np.zeros(BATCH, dtype=np.float64)
    for cid in range(N_CORES):
        ope = np.asarray(results[cid]["out_pe"], dtype=np.float64).reshape(-1)
        odv = np.asarray(results[cid]["out_dve"], dtype=np.float64)
        for e, (kind, w, s) in enumerate(EMIT):
            base = batch_base(kind, w, s)
            out[base:base + 512] += ope[e * 512:(e + 1) * 512]
        out[B_PE:] += odv.T.reshape(-1)
    return out.astype(np.float32)


def kernel(x: np.ndarray, weight: np.ndarray) -> np.ndarray:
    x = np.asarray(x, dtype=np.float32)
    weight = np.asarray(weight, dtype=np.float32)
    assert x.shape == (BATCH, IN_SIZE) and weight.shape == (W_ROWS, IN_SIZE)
    nc = _get_nc()
    res = run_bass_kernel_spmd(nc, make_in_maps(x, weight), list(range(N_CORES))).results
    return assemble(res)


# revision 14
# speedup vs baseline: 2.1987x; 1.0108x over previous
"""Trainium2 Bass kernel for: out = SCALE * x @ weight.sum(axis=0).

Column-sharded over 8 cores (stripe of 512 cols each); every core computes
partial dot products for ALL 16384 batch rows over its stripe; host sums the
8 partials. All device inputs bf16 (host casts during sharding; tolerance is
2e-2 so bf16 inputs with fp32 accumulation are well within budget).

Per-core pipeline:
  Phase W (wsum = colsum of the weight stripe):
    - w_nat [W_PE, 512] natural rows-share, one DMA per 128-row block so PE
      ones-matmuls start with the first block: row partial [1, 512] in PSUM
    - w_t [512, W_T] transposed rows-share, split by column chunk:
      ACT accum_out on chunks 0-1, DVE tensor_reduce on chunks 2-3
      -> column-form partials [128, 4] (f32)
    - fold (PE transposes with identity): col->row, add, broadcast row to
      all partitions via ones-matmul (wsumB), and row->col lhsT (wcolT bf16)
  Phase X:
    - PE share: transposed x windows, 4 chunk-matmuls per 512-batch group
      into a PSUM bank slot (partitions 0/32/64); slots evicted with SCALE,
      split between ACT and DVE to balance queue occupancy
    - DVE share: natural x, fused scalar_tensor_tensor per 128-row block
  DMA: three plain-DMA streams (sync/scalar HWDGE ~350 B/ns each, gpsimd
  SWDGE ~330 B/ns); a DMA occupies its issuing queue for the transfer, so
  scalar-queue DMAs are budgeted around ACT compute.
"""

import numpy as np
import ml_dtypes

from concourse import bacc, bass, tile
import concourse.mybir as mybir
from concourse.bass_utils import run_bass_kernel_spmd

BF16NP = ml_dtypes.bfloat16

N_CORES = 8
BATCH = 16384
IN_SIZE = 4096
CS = IN_SIZE // N_CORES  # 512 cols per core
W_ROWS = 4096
SCALE = 0.5
P = 128

# --- tunables ---------------------------------------------------------------
B_PE = 11776            # PE batch rows: 23 groups of 512
B_DVE = BATCH - B_PE    # 4608 -> 36 DVE blocks
W_PE = 2048             # weight rows summed by PE (natural), 16 blocks
W_T = W_ROWS - W_PE     # 2048 transposed rows: chunks 0-1 ACT, 2-3 DVE
N_DBLK = B_DVE // P     # 36 DVE blocks
N_DTILE = N_DBLK // 4   # 9 tiles of [128, 4, 512]
N_GROUPS = B_PE // 512  # 23 PSUM groups

# x_pe windows: (batch_cols, queue) in emission order; sum = B_PE
XPE_WINDOWS = [
    (2048, "pool"), (1536, "scalar"), (2048, "pool"), (2048, "sync"),
    (2048, "pool"), (2048, "scalar"),
]
assert sum(w for w, _ in XPE_WINDOWS) == B_PE

bf16 = mybir.dt.bfloat16
fp32 = mybir.dt.float32
i16 = mybir.dt.int16


def build_nc(for_sim: bool = False):
    if for_sim:
        nc = bacc.Bacc(None, target_bir_lowering=False, debug=True, num_devices=1)
    else:
        nc = bacc.Bacc(None, num_devices=N_CORES)

    x_pe_t = nc.declare_dram_parameter("x_pe_t", [CS, B_PE], bf16, isOutput=False)
    x_dve = nc.declare_dram_parameter("x_dve", [B_DVE, CS], bf16, isOutput=False)
    w_nat = nc.declare_dram_parameter("w_nat", [W_PE, CS], bf16, isOutput=False)
    w_t = nc.declare_dram_parameter("w_t", [CS, W_T], bf16, isOutput=False)
    ident_ext = nc.declare_dram_parameter("ident", [P, P], fp32, isOutput=False)
    out_pe = nc.declare_dram_parameter("out_pe", [3, 4096], fp32, isOutput=True)
    out_dve = nc.declare_dram_parameter("out_dve", [P, N_DBLK], fp32, isOutput=True)

    with tile.TileContext(nc) as tc:
        with (
            tc.tile_pool(name="xw", bufs=6) as xw_pool,
            tc.tile_pool(name="xd", bufs=9) as xd_pool,
            tc.tile_pool(name="wpool", bufs=1) as wpool,
            tc.tile_pool(name="aux", bufs=1) as aux,
            tc.tile_pool(name="psum", bufs=1, space="PSUM") as psum,
        ):
            qmap = {"sync": nc.sync, "scalar": nc.scalar, "pool": nc.gpsimd}

            # --- Phase W DMAs: w_nat per-block first (PE), then w_t ---------
            wb_tiles = []
            n_wb = W_PE // P  # 16 blocks, alternate sync/scalar
            for b in range(n_wb):
                wb = wpool.tile([P, CS], bf16, tag=f"wb{b}")
                eng = nc.sync if b % 2 == 0 else nc.scalar
                eng.dma_start(out=wb[:], in_=w_nat[b * P:(b + 1) * P, :])
                wb_tiles.append(wb)
            ident = aux.tile([P, P], fp32)
            nc.scalar.dma_start(out=ident[:], in_=ident_ext[:, :])
            wt_tiles = []
            for c in range(4):
                wt_c = wpool.tile([P, W_T], bf16, tag=f"wt{c}")
                (nc.sync if c % 2 == 0 else nc.scalar).dma_start(
                    out=wt_c[:], in_=w_t[c * P:(c + 1) * P, :])
                wt_tiles.append(wt_c)

            # --- Phase W compute -------------------------------------------
            ones = aux.tile([P, 1], bf16)
            nc.vector.memset(ones[:], 1.0)
            onesr = aux.tile([1, P], fp32)
            nc.vector.memset(onesr[:], 1.0)

            pw = psum.tile([P, 4096], fp32)
            for b in range(n_wb):
                nc.tensor.matmul(
                    pw[0:1, 0:CS], ones[:], wb_tiles[b][:],
                    start=(b == 0), stop=(b == n_wb - 1))

            wcol = aux.tile([P, 4], fp32)
            for c in range(2):  # ACT chunks
                nc.scalar.activation(
                    out=wt_tiles[c][:], in_=wt_tiles[c][:],
                    func=mybir.ActivationFunctionType.Copy,
                    accum_out=wcol[:, c:c + 1])
            for c in range(2, 4):  # DVE chunks
                nc.vector.tensor_reduce(
                    out=wcol[:, c:c + 1], in_=wt_tiles[c][:],
                    axis=mybir.AxisListType.X, op=mybir.AluOpType.add)

            # fold: col partials -> row (PE transposes into psum bank1), add
            for c in range(4):
                nc.tensor.matmul(
                    pw[0:1, 512 + c * P: 512 + (c + 1) * P],
                    wcol[:, c:c + 1], ident[:, :],
                    is_transpose=True, start=True, stop=True)
            wrow = aux.tile([1, CS], fp32)
            nc.vector.tensor_tensor(
                out=wrow[:], in0=pw[0:1, 0:CS], in1=pw[0:1, 512:1024],
                op=mybir.AluOpType.add)
            # broadcast row to all partitions via ones-matmul (PSUM bank 3)
            nc.tensor.matmul(
                pw[0:P, 1536:2048], onesr[:], wrow[:], start=True, stop=True)
            wsumB = aux.tile([P, CS], fp32)
            nc.vector.tensor_copy(wsumB[:], pw[0:P, 1536:2048])
            # row -> col lhsT form
            for c in range(4):
                nc.tensor.matmul(
                    pw[0:P, 1024 + c: 1024 + c + 1],
                    wrow[0:1, c * P:(c + 1) * P], ident[0:1, 0:1],
                    is_transpose=True, start=True, stop=True)
            wcolT_bf = aux.tile([P, 4], bf16)
            nc.vector.tensor_copy(wcolT_bf[:], pw[0:P, 1024:1028])

            # --- Phase X DMAs ----------------------------------------------
            xw_tiles = []
            off = 0
            for i, (wlen, q) in enumerate(XPE_WINDOWS):
                xt = xw_pool.tile([P, 4, wlen], bf16, tag="xw")
                qmap[q].dma_start(
                    out=xt[:],
                    in_=x_pe_t[:, off:off + wlen].rearrange(
                        "(c p) f -> p c f", p=P))
                xw_tiles.append((xt, wlen, off))
                off += wlen

            xd_tiles = []
            for t in range(N_DTILE):
                xd = xd_pool.tile([P, 4, CS], bf16, tag="xd")
                nc.sync.dma_start(
                    out=xd[:],
                    in_=x_dve[t * 4 * P:(t + 1) * 4 * P, :].rearrange(
                        "(g p) f -> p g f", p=P))
                xd_tiles.append(xd)

            # --- Phase X compute -------------------------------------------
            osb_pe = aux.tile([P, 4096], fp32)
            osb_dve = aux.tile([P, N_DBLK], fp32)

            dve_t = 0

            def emit_dve_tile():
                nonlocal dve_t
                if dve_t >= N_DTILE:
                    return
                xd = xd_tiles[dve_t]
                for gi in range(4):
                    col = dve_t * 4 + gi
                    nc.vector.scalar_tensor_tensor(
                        out=xd[:, gi, :], in0=xd[:, gi, :], scalar=SCALE,
                        in1=wsumB[:],
                        op0=mybir.AluOpType.mult, op1=mybir.AluOpType.mult,
                        accum_out=osb_dve[:, col:col + 1])
                dve_t += 1

            e = 0
            for xt, wlen, off in xw_tiles:
                for s in range(wlen // 512):
                    bank, slot = e % 8, e // 8
                    ps = pw[slot * 32: slot * 32 + 1,
                            bank * 512:(bank + 1) * 512]
                    for c in range(4):
                        nc.tensor.matmul(
                            ps, wcolT_bf[:, c:c + 1],
                            xt[:, c, s * 512:(s + 1) * 512],
                            start=(c == 0), stop=(c == 3))
                    if e == 15:
                        # slots 0+1 full: one ACT pass
                        nc.scalar.activation(
                            out=osb_pe[0:33:32, :], in_=pw[0:33:32, :],
                            func=mybir.ActivationFunctionType.Copy, scale=SCALE)
                    elif e == 19:
                        nc.vector.tensor_scalar(
                            out=osb_pe[64:65, 0:2048], in0=pw[64:65, 0:2048],
                            scalar1=SCALE, scalar2=None,
                            op0=mybir.AluOpType.mult)
                    elif e == 22:
                        nc.scalar.activation(
                            out=osb_pe[64:65, 2048:3584],
                            in_=pw[64:65, 2048:3584],
                            func=mybir.ActivationFunctionType.Copy, scale=SCALE)
                    e += 1
                    if e % 2 == 0:
                        emit_dve_tile()
            while dve_t < N_DTILE:
                emit_dve_tile()

            for s in range(3):
                ncols = 4096 if s < 2 else 3584
                nc.gpsimd.dma_start(
                    out=out_pe[s:s + 1, 0:ncols],
                    in_=osb_pe[s * 32:s * 32 + 1, 0:ncols])
            nc.gpsimd.dma_start(out=out_dve[:], in_=osb_dve[:])

    return nc


_NC_CACHE: dict = {}


def _get_nc():
    if "nc" not in _NC_CACHE:
        nc = build_nc()
        nc.finalize()
        _NC_CACHE["nc"] = nc
    return _NC_CACHE["nc"]


def make_in_maps(x: np.ndarray, weight: np.ndarray):
    ident = np.eye(P, dtype=np.float32)
    maps = []
    for c in range(N_CORES):
        sl = slice(c * CS, (c + 1) * CS)
        xs = x[:, sl].astype(BF16NP)
        ws = weight[:, sl].astype(BF16NP)
        maps.append({
            "x_pe_t": np.ascontiguousarray(xs[:B_PE].T),
            "x_dve": np.ascontiguousarray(xs[B_PE:]),
            "w_nat": np.ascontiguousarray(ws[:W_PE]),
            "w_t": np.ascontiguousarray(ws[W_PE:].T),
            "ident": ident,
        })
    return maps


def assemble(results) -> np.ndarray:
    out = np.zeros(BATCH, dtype=np.float64)
    for cid in range(N_CORES):
        ope = np.asarray(results[cid]["out_pe"], dtype=np.float64).reshape(-1)
        odv = np.asarray(results[cid]["out_dve"], dtype=np.float64)
        out[:B_PE] += ope[:B_PE]
        out[B_PE:] += odv.T.reshape(-1)
    return out.astype(np.float32)


def kernel(x: np.ndarray, weight: np.ndarray) -> np.ndarray:
    x = np.asarray(x, dtype=np.float32)
    weight = np.asarray(weight, dtype=np.float32)
    assert x.shape == (BATCH, IN_SIZE) and weight.shape == (W_ROWS, IN_SIZE)
    nc = _get_nc()
    res = run_bass_kernel_spmd(nc, make_in_maps(x, weight), list(range(N_CORES))).results
    return assemble(res)


# revision 16
# speedup vs baseline: 2.4168x; 1.0992x over previous
"""Trainium2 Bass kernel for: out = SCALE * x @ weight.sum(axis=0).

Column-sharded over 8 cores (stripe of 512 cols each); every core computes
partial dot products for ALL 16384 batch rows over its stripe; host sums the
8 partials. All device inputs bf16 (host casts during sharding; tolerance is
2e-2 so bf16 inputs with fp32 accumulation are well within budget).

Per-core pipeline:
  Phase W (wsum = colsum of the weight stripe):
    - w_nat [W_PE, 512] natural rows-share, paired-block DMAs so PE
      ones-matmuls start with the first arrival: row partial [1,512] in PSUM
    - w_t [512, W_T] transposed rows-share split by column chunk:
      ACT accum_out (chunks 0,2) + DVE tensor_reduce (chunks 1,3)
      -> column partials [128, 4] f32
    - fold via PE transpose-matmuls (identity input): col->row, DVE add ->
      wrow; ones-matmul broadcast -> wsumB (all partitions); 4 transposes ->
      wcolT bf16 (PE lhsT)
  Phase X:
    - PE: transposed x windows, 4 chunk-matmuls per 512-batch group into
      PSUM partition-slots; two dedicated PSUM tiles (12 groups each) so
      evictions never false-conflict with later matmuls; ACT evicts with
      SCALE using 4-partition strided reads (cheap, free-size driven)
    - DVE: natural x, fused scalar_tensor_tensor per 128-row block
  DMA: three plain-DMA queues (sync/scalar HWDGE, gpsimd SWDGE), each
  ~330-350 B/ns, transfers occupying their issuing queue; windows are
  spread across queues so PE receives a steady interleaved feed.
"""

import numpy as np
import ml_dtypes

from concourse import bacc, bass, tile
import concourse.mybir as mybir
from concourse.bass_utils import run_bass_kernel_spmd

BF16NP = ml_dtypes.bfloat16

N_CORES = 8
BATCH = 16384
IN_SIZE = 4096
CS = IN_SIZE // N_CORES  # 512
W_ROWS = 4096
SCALE = 0.5
P = 128

# --- tunables ---------------------------------------------------------------
B_PE = 12288            # 24 PSUM groups of 512
B_DVE = BATCH - B_PE    # 4096 -> 32 DVE blocks
W_PE = 2048             # 16 natural blocks (8 pair-DMAs)
W_T = W_ROWS - W_PE     # 2048 transposed rows
N_DBLK = B_DVE // P     # 32
N_DTILE = N_DBLK // 4   # 8 tiles [128, 4, 512]

# x_pe windows: (cols, queue); emission below interleaves expected arrivals
XPE_WINDOWS = [
    (2048, "pool"),    # xw0
    (2048, "sync"),    # xwC
    (2048, "scalar"),  # xwB
    (2048, "pool"),    # xw1
    (2048, "scalar"),  # xwE
    (2048, "pool"),    # xw2
]
assert sum(w for w, _ in XPE_WINDOWS) == B_PE

bf16 = mybir.dt.bfloat16
fp32 = mybir.dt.float32


def build_nc(for_sim: bool = False):
    if for_sim:
        nc = bacc.Bacc(None, target_bir_lowering=False, debug=True, num_devices=1)
    else:
        nc = bacc.Bacc(None, num_devices=N_CORES)

    x_pe_t = nc.declare_dram_parameter("x_pe_t", [CS, B_PE], bf16, isOutput=False)
    x_dve = nc.declare_dram_parameter("x_dve", [B_DVE, CS], bf16, isOutput=False)
    w_nat = nc.declare_dram_parameter("w_nat", [W_PE, CS], bf16, isOutput=False)
    w_t = nc.declare_dram_parameter("w_t", [CS, W_T], bf16, isOutput=False)
    ident_ext = nc.declare_dram_parameter("ident", [P, P], fp32, isOutput=False)
    out_pe = nc.declare_dram_parameter("out_pe", [6, 2048], fp32, isOutput=True)
    out_dve = nc.declare_dram_parameter("out_dve", [P, N_DBLK], fp32, isOutput=True)

    with tile.TileContext(nc) as tc:
        with (
            tc.tile_pool(name="xw", bufs=6) as xw_pool,
            tc.tile_pool(name="xd", bufs=8) as xd_pool,
            tc.tile_pool(name="wpool", bufs=1) as wpool,
            tc.tile_pool(name="aux", bufs=1) as aux,
            tc.tile_pool(name="psum", bufs=1, space="PSUM") as psum,
        ):
            qmap = {"sync": nc.sync, "scalar": nc.scalar, "pool": nc.gpsimd}

            # --- Phase W DMAs ----------------------------------------------
            # transposed chunks: c0 (ACT) and c1 (DVE) on sync; c2 (ACT) and
            # c3 (DVE) on scalar / pool
            wt_tiles = []
            for c, q in ((0, "sync"), (1, "sync"), (2, "scalar"), (3, "pool")):
                wt_c = wpool.tile([P, W_T], bf16, tag=f"wt{c}")
                qmap[q].dma_start(out=wt_c[:], in_=w_t[c * P:(c + 1) * P, :])
                wt_tiles.append(wt_c)
            ident = aux.tile([P, P], fp32)
            nc.scalar.dma_start(out=ident[:], in_=ident_ext[:, :])
            # natural blocks, pairs alternating sync/scalar
            wb_tiles = []
            n_pairs = W_PE // (2 * P)  # 8
            for pr in range(n_pairs):
                wb = wpool.tile([P, 2, CS], bf16, tag=f"wb{pr}")
                eng = nc.sync if pr % 2 == 0 else nc.scalar
                eng.dma_start(
                    out=wb[:],
                    in_=w_nat[pr * 2 * P:(pr + 1) * 2 * P, :].rearrange(
                        "(g p) f -> p g f", p=P))
                wb_tiles.append(wb)

            # --- Phase W compute -------------------------------------------
            ones = aux.tile([P, 1], bf16)
            nc.vector.memset(ones[:], 1.0)
            onesr = aux.tile([1, P], fp32)
            nc.vector.memset(onesr[:], 1.0)

            # one PSUM tile; banks 0-3 hold groups 0-11 (A), banks 4-7
            # groups 12-23 (B) -> byte-disjoint so evictA never blocks B
            # matmuls. W scratch reuses banks 0-1 before the A groups start.
            psX = psum.tile([P, 4096], fp32)

            n_wb = W_PE // P
            for b in range(n_wb):
                nc.tensor.matmul(
                    psX[0:1, 0:CS], ones[:], wb_tiles[b // 2][:, b % 2, :],
                    start=(b == 0), stop=(b == n_wb - 1))

            wcol = aux.tile([P, 4], fp32)
            for c in (0, 2):  # ACT chunks
                nc.scalar.activation(
                    out=wt_tiles[c][:], in_=wt_tiles[c][:],
                    func=mybir.ActivationFunctionType.Copy,
                    accum_out=wcol[:, c:c + 1])
            for c in (1, 3):  # DVE chunks
                nc.vector.tensor_reduce(
                    out=wcol[:, c:c + 1], in_=wt_tiles[c][:],
                    axis=mybir.AxisListType.X, op=mybir.AluOpType.add)

            # fold
            for c in range(4):
                nc.tensor.matmul(
                    psX[0:1, 512 + c * P: 512 + (c + 1) * P],
                    wcol[:, c:c + 1], ident[:, :],
                    is_transpose=True, start=True, stop=True)
            wrow = aux.tile([1, CS], fp32)
            nc.vector.tensor_tensor(
                out=wrow[:], in0=psX[0:1, 0:CS], in1=psX[0:1, 512:1024],
                op=mybir.AluOpType.add)
            # broadcast first (unblocks DVE), then lhsT form (unblocks PE)
            nc.tensor.matmul(psX[0:P, 0:512], onesr[:], wrow[:], start=True, stop=True)
            wsumB = aux.tile([P, CS], fp32)
            nc.vector.tensor_copy(wsumB[:], psX[0:P, 0:512])
            for c in range(4):
                nc.tensor.matmul(
                    psX[0:P, 512 + c: 512 + c + 1],
                    wrow[0:1, c * P:(c + 1) * P], ident[0:1, 0:1],
                    is_transpose=True, start=True, stop=True)
            wcolT_bf = aux.tile([P, 4], bf16)
            nc.vector.tensor_copy(wcolT_bf[:], psX[0:P, 512:516])

            # --- Phase X DMAs ----------------------------------------------
            xw_tiles = []
            off = 0
            for wlen, q in XPE_WINDOWS:
                xt = xw_pool.tile([P, 4, wlen], bf16, tag="xw")
                qmap[q].dma_start(
                    out=xt[:],
                    in_=x_pe_t[:, off:off + wlen].rearrange(
                        "(c p) f -> p c f", p=P))
                xw_tiles.append((xt, wlen))
                off += wlen

            xd_tiles = []
            for t in range(N_DTILE):
                xd = xd_pool.tile([P, 4, CS], bf16, tag="xd")
                eng = nc.gpsimd if t == N_DTILE - 1 else nc.sync
                eng.dma_start(
                    out=xd[:],
                    in_=x_dve[t * 4 * P:(t + 1) * 4 * P, :].rearrange(
                        "(g p) f -> p g f", p=P))
                xd_tiles.append(xd)

            # --- Phase X compute -------------------------------------------
            osb_pe = aux.tile([P, 4096], fp32)
            osb_dve = aux.tile([P, N_DBLK], fp32)

            dve_t = 0

            def emit_dve_tile():
                nonlocal dve_t
                if dve_t >= N_DTILE:
                    return
                xd = xd_tiles[dve_t]
                for gi in range(4):
                    col = dve_t * 4 + gi
                    nc.vector.scalar_tensor_tensor(
                        out=xd[:, gi, :], in0=xd[:, gi, :], scalar=SCALE,
                        in1=wsumB[:],
                        op0=mybir.AluOpType.mult, op1=mybir.AluOpType.mult,
                        accum_out=osb_dve[:, col:col + 1])
                dve_t += 1

            e = 0
            for xt, wlen in xw_tiles:
                for s in range(wlen // 512):
                    half, eb = divmod(e, 12)
                    bank, slot = half * 4 + eb % 4, eb // 4
                    ps = psX[slot * 32: slot * 32 + 1,
                             bank * 512:(bank + 1) * 512]
                    for c in range(4):
                        nc.tensor.matmul(
                            ps, wcolT_bf[:, c:c + 1],
                            xt[:, c, s * 512:(s + 1) * 512],
                            start=(c == 0), stop=(c == 3))
                    if e == 11:
                        nc.scalar.activation(
                            out=osb_pe[0:65:32, 0:2048], in_=psX[0:65:32, 0:2048],
                            func=mybir.ActivationFunctionType.Copy, scale=SCALE)
                    elif e == 23:
                        nc.scalar.activation(
                            out=osb_pe[0:65:32, 2048:4096], in_=psX[0:65:32, 2048:4096],
                            func=mybir.ActivationFunctionType.Copy, scale=SCALE)
                    e += 1
                    if e % 3 == 0:
                        emit_dve_tile()
            while dve_t < N_DTILE:
                emit_dve_tile()

            for r in range(6):
                half, slot = divmod(r, 3)
                nc.gpsimd.dma_start(
                    out=out_pe[r:r + 1, :],
                    in_=osb_pe[slot * 32:slot * 32 + 1,
                               half * 2048:(half + 1) * 2048])
            nc.gpsimd.dma_start(out=out_dve[:], in_=osb_dve[:])

    return nc


_NC_CACHE: dict = {}


def _get_nc():
    if "nc" not in _NC_CACHE:
        nc = build_nc()
        nc.finalize()
        _NC_CACHE["nc"] = nc
    return _NC_CACHE["nc"]


def make_in_maps(x: np.ndarray, weight: np.ndarray):
    ident = np.eye(P, dtype=np.float32)
    maps = []
    for c in range(N_CORES):
        sl = slice(c * CS, (c + 1) * CS)
        xs = x[:, sl].astype(BF16NP)
        ws = weight[:, sl].astype(BF16NP)
        maps.append({
            "x_pe_t": np.ascontiguousarray(xs[:B_PE].T),
            "x_dve": np.ascontiguousarray(xs[B_PE:]),
            "w_nat": np.ascontiguousarray(ws[:W_PE]),
            "w_t": np.ascontiguousarray(ws[W_PE:].T),
            "ident": ident,
        })
    return maps


def pe_batch_order(ope_flat: np.ndarray) -> np.ndarray:
    """out_pe [6, 2048] -> batch-ordered [B_PE]. Row r=half*3+slot holds
    4 banks of that half's slot; group e = half*12 + slot*4 + bank."""
    out = np.empty(B_PE, dtype=ope_flat.dtype)
    ope = ope_flat.reshape(6, 4, 512)
    for half in range(2):
        for slot in range(3):
            for bank in range(4):
                e = half * 12 + slot * 4 + bank
                out[e * 512:(e + 1) * 512] = ope[half * 3 + slot, bank]
    return out


def assemble(results) -> np.ndarray:
    out = np.zeros(BATCH, dtype=np.float64)
    for cid in range(N_CORES):
        ope = np.asarray(results[cid]["out_pe"], dtype=np.float64)
        odv = np.asarray(results[cid]["out_dve"], dtype=np.float64)
        out[:B_PE] += pe_batch_order(ope)
        out[B_PE:] += odv.T.reshape(-1)
    return out.astype(np.float32)


def kernel(x: np.ndarray, weight: np.ndarray) -> np.ndarray:
    x = np.asarray(x, dtype=np.float32)
    weight = np.asarray(weight, dtype=np.float32)
    assert x.shape == (BATCH, IN_SIZE) and weight.shape == (W_ROWS, IN_SIZE)
    nc = _get_nc()
    res = run_bass_kernel_spmd(nc, make_in_maps(x, weight), list(range(N_CORES))).results
    return assemble(res)


# revision 17
# speedup vs baseline: 2.4471x; 1.0125x over previous
"""Trainium2 Bass kernel for: out = SCALE * x @ weight.sum(axis=0).

Column-sharded over 8 cores (stripe of 512 cols each); every core computes
partial dot products for ALL 16384 batch rows over its stripe; host sums the
8 partials. All device inputs bf16 (host casts during sharding; tolerance is
2e-2 so bf16 inputs with fp32 accumulation are well within budget).

Per-core pipeline:
  Phase W (wsum = colsum of the weight stripe):
    - w_nat [W_PE, 512] natural rows-share, paired-block DMAs so PE
      ones-matmuls start with the first arrival: row partial [1,512] in PSUM
    - w_t [512, W_T] transposed rows-share split by column chunk:
      ACT accum_out (chunks 0,2) + DVE tensor_reduce (chunks 1,3)
      -> column partials [128, 4] f32
    - fold via PE transpose-matmuls (identity input): col->row, DVE add ->
      wrow; ones-matmul broadcast -> wsumB (all partitions); 4 transposes ->
      wcolT bf16 (PE lhsT)
  Phase X:
    - PE: transposed x windows, 4 chunk-matmuls per 512-batch group into
      PSUM partition-slots; two dedicated PSUM tiles (12 groups each) so
      evictions never false-conflict with later matmuls; ACT evicts with
      SCALE using 4-partition strided reads (cheap, free-size driven)
    - DVE: natural x, fused scalar_tensor_tensor per 128-row block
  DMA: three plain-DMA queues (sync/scalar HWDGE, gpsimd SWDGE), each
  ~330-350 B/ns, transfers occupying their issuing queue; windows are
  spread across queues so PE receives a steady interleaved feed.
"""

import numpy as np
import ml_dtypes

from concourse import bacc, bass, tile
import concourse.mybir as mybir
from concourse.bass_utils import run_bass_kernel_spmd

BF16NP = ml_dtypes.bfloat16

N_CORES = 8
BATCH = 16384
IN_SIZE = 4096
CS = IN_SIZE // N_CORES  # 512
W_ROWS = 4096
SCALE = 0.5
P = 128

# --- tunables ---------------------------------------------------------------
B_PE = 12288            # 24 PSUM groups of 512
B_DVE = BATCH - B_PE    # 4096 -> 32 DVE blocks
W_PE = 3072             # 24 natural blocks (12 pair-DMAs)
W_T = W_ROWS - W_PE     # 1024 transposed rows
N_DBLK = B_DVE // P     # 32
N_DTILE = N_DBLK // 4   # 8 tiles [128, 4, 512]

# x_pe windows: (cols, queue); emission below interleaves expected arrivals
XPE_WINDOWS = [
    (2048, "pool"),    # xw0
    (2048, "sync"),    # xwC
    (2048, "scalar"),  # xwB
    (2048, "pool"),    # xw1
    (2048, "scalar"),  # xwE
    (2048, "pool"),    # xw2
]
assert sum(w for w, _ in XPE_WINDOWS) == B_PE

bf16 = mybir.dt.bfloat16
fp32 = mybir.dt.float32


def build_nc(for_sim: bool = False):
    if for_sim:
        nc = bacc.Bacc(None, target_bir_lowering=False, debug=True, num_devices=1)
    else:
        nc = bacc.Bacc(None, num_devices=N_CORES)

    x_pe_t = nc.declare_dram_parameter("x_pe_t", [CS, B_PE], bf16, isOutput=False)
    x_dve = nc.declare_dram_parameter("x_dve", [B_DVE, CS], bf16, isOutput=False)
    w_nat = nc.declare_dram_parameter("w_nat", [W_PE, CS], bf16, isOutput=False)
    w_t = nc.declare_dram_parameter("w_t", [CS, W_T], bf16, isOutput=False)
    ident_ext = nc.declare_dram_parameter("ident", [P, P], fp32, isOutput=False)
    out_pe = nc.declare_dram_parameter("out_pe", [6, 2048], fp32, isOutput=True)
    out_dve = nc.declare_dram_parameter("out_dve", [P, N_DBLK], fp32, isOutput=True)

    with tile.TileContext(nc) as tc:
        with (
            tc.tile_pool(name="xw", bufs=6) as xw_pool,
            tc.tile_pool(name="xd", bufs=8) as xd_pool,
            tc.tile_pool(name="wpool", bufs=1) as wpool,
            tc.tile_pool(name="aux", bufs=1) as aux,
            tc.tile_pool(name="psum", bufs=1, space="PSUM") as psum,
        ):
            qmap = {"sync": nc.sync, "scalar": nc.scalar, "pool": nc.gpsimd}

            # --- Phase W DMAs ----------------------------------------------
            # transposed chunks: c0/c2 -> ACT, c1/c3 -> DVE
            wt_tiles = [None] * 4
            for c, q in ((0, "sync"), (1, "sync"), (2, "scalar"), (3, "pool")):
                wt_c = wpool.tile([P, W_T], bf16, tag=f"wt{c}")
                qmap[q].dma_start(out=wt_c[:], in_=w_t[c * P:(c + 1) * P, :])
                wt_tiles[c] = wt_c
            ident = aux.tile([P, P], fp32)
            nc.scalar.dma_start(out=ident[:], in_=ident_ext[:, :])
            # natural blocks: 12 pairs over sync(6)/scalar(3)/pool(3)
            wb_tiles = []
            n_pairs = W_PE // (2 * P)
            pair_q = ["sync", "scalar", "pool", "sync", "scalar", "pool",
                      "sync", "sync", "scalar", "sync", "pool", "sync"]
            for pr in range(n_pairs):
                wb = wpool.tile([P, 2, CS], bf16, tag=f"wb{pr}")
                qmap[pair_q[pr]].dma_start(
                    out=wb[:],
                    in_=w_nat[pr * 2 * P:(pr + 1) * 2 * P, :].rearrange(
                        "(g p) f -> p g f", p=P))
                wb_tiles.append(wb)

            # --- Phase W compute -------------------------------------------
            ones = aux.tile([P, 1], bf16)
            nc.vector.memset(ones[:], 1.0)
            onesr = aux.tile([1, P], fp32)
            nc.vector.memset(onesr[:], 1.0)

            # one PSUM tile; banks 0-3 hold groups 0-11 (A), banks 4-7
            # groups 12-23 (B) -> byte-disjoint so evictA never blocks B
            # matmuls. W scratch reuses banks 0-1 before the A groups start.
            psX = psum.tile([P, 4096], fp32)

            n_wb = W_PE // P
            for b in range(n_wb):
                nc.tensor.matmul(
                    psX[0:1, 0:CS], ones[:], wb_tiles[b // 2][:, b % 2, :],
                    start=(b == 0), stop=(b == n_wb - 1))

            wcol = aux.tile([P, 4], fp32)
            for c in (0, 2):  # ACT chunks
                nc.scalar.activation(
                    out=wt_tiles[c][:], in_=wt_tiles[c][:],
                    func=mybir.ActivationFunctionType.Copy,
                    accum_out=wcol[:, c:c + 1])
            for c in (1, 3):  # DVE chunks
                nc.vector.tensor_reduce(
                    out=wcol[:, c:c + 1], in_=wt_tiles[c][:],
                    axis=mybir.AxisListType.X, op=mybir.AluOpType.add)

            # fold
            for c in range(4):
                nc.tensor.matmul(
                    psX[0:1, 512 + c * P: 512 + (c + 1) * P],
                    wcol[:, c:c + 1], ident[:, :],
                    is_transpose=True, start=True, stop=True)
            wrow = aux.tile([1, CS], fp32)
            nc.vector.tensor_tensor(
                out=wrow[:], in0=psX[0:1, 0:CS], in1=psX[0:1, 512:1024],
                op=mybir.AluOpType.add)
            # broadcast first (unblocks DVE), then lhsT form (unblocks PE)
            nc.tensor.matmul(psX[0:P, 0:512], onesr[:], wrow[:], start=True, stop=True)
            wsumB = aux.tile([P, CS], fp32)
            nc.scalar.activation(
                out=wsumB[:], in_=psX[0:P, 0:512],
                func=mybir.ActivationFunctionType.Copy)
            for c in range(4):
                nc.tensor.matmul(
                    psX[0:P, 512 + c: 512 + c + 1],
                    wrow[0:1, c * P:(c + 1) * P], ident[0:1, 0:1],
                    is_transpose=True, start=True, stop=True)
            wcolT_bf = aux.tile([P, 4], bf16)
            nc.vector.tensor_copy(wcolT_bf[:], psX[0:P, 512:516])

            # --- Phase X DMAs ----------------------------------------------
            xw_tiles = []
            off = 0
            for wlen, q in XPE_WINDOWS:
                xt = xw_pool.tile([P, 4, wlen], bf16, tag="xw")
                qmap[q].dma_start(
                    out=xt[:],
                    in_=x_pe_t[:, off:off + wlen].rearrange(
                        "(c p) f -> p c f", p=P))
                xw_tiles.append((xt, wlen))
                off += wlen

            xd_tiles = []
            for t in range(N_DTILE):
                xd = xd_pool.tile([P, 4, CS], bf16, tag="xd")
                eng = nc.gpsimd if t == N_DTILE - 1 else nc.sync
                eng.dma_start(
                    out=xd[:],
                    in_=x_dve[t * 4 * P:(t + 1) * 4 * P, :].rearrange(
                        "(g p) f -> p g f", p=P))
                xd_tiles.append(xd)

            # --- Phase X compute -------------------------------------------
            osb_pe = aux.tile([P, 4096], fp32)
            osb_dve = aux.tile([P, N_DBLK], fp32)

            dve_t = 0

            def emit_dve_tile():
                nonlocal dve_t
                if dve_t >= N_DTILE:
                    return
                xd = xd_tiles[dve_t]
                for gi in range(4):
                    col = dve_t * 4 + gi
                    nc.vector.scalar_tensor_tensor(
                        out=xd[:, gi, :], in0=xd[:, gi, :], scalar=SCALE,
                        in1=wsumB[:],
                        op0=mybir.AluOpType.mult, op1=mybir.AluOpType.mult,
                        accum_out=osb_dve[:, col:col + 1])
                dve_t += 1

            e = 0
            for xt, wlen in xw_tiles:
                for s in range(wlen // 512):
                    half, eb = divmod(e, 12)
                    bank, slot = half * 4 + eb % 4, eb // 4
                    ps = psX[slot * 32: slot * 32 + 1,
                             bank * 512:(bank + 1) * 512]
                    for c in range(4):
                        nc.tensor.matmul(
                            ps, wcolT_bf[:, c:c + 1],
                            xt[:, c, s * 512:(s + 1) * 512],
                            start=(c == 0), stop=(c == 3))
                    if e == 11:
                        nc.scalar.activation(
                            out=osb_pe[0:65:32, 0:2048], in_=psX[0:65:32, 0:2048],
                            func=mybir.ActivationFunctionType.Copy, scale=SCALE)
                    elif e == 23:
                        nc.scalar.activation(
                            out=osb_pe[0:65:32, 2048:4096], in_=psX[0:65:32, 2048:4096],
                            func=mybir.ActivationFunctionType.Copy, scale=SCALE)
                    e += 1
                    if e % 3 == 0:
                        emit_dve_tile()
            while dve_t < N_DTILE:
                emit_dve_tile()

            for r in range(6):
                half, slot = divmod(r, 3)
                nc.gpsimd.dma_start(
                    out=out_pe[r:r + 1, :],
                    in_=osb_pe[slot * 32:slot * 32 + 1,
                               half * 2048:(half + 1) * 2048])
            nc.gpsimd.dma_start(out=out_dve[:], in_=osb_dve[:])

    return nc


_NC_CACHE: dict = {}


def _get_nc():
    if "nc" not in _NC_CACHE:
        nc = build_nc()
        nc.finalize()
        _NC_CACHE["nc"] = nc
    return _NC_CACHE["nc"]


def make_in_maps(x: np.ndarray, weight: np.ndarray):
    ident = np.eye(P, dtype=np.float32)
    maps = []
    for c in range(N_CORES):
        sl = slice(c * CS, (c + 1) * CS)
        xs = x[:, sl].astype(BF16NP)
        ws = weight[:, sl].astype(BF16NP)
        maps.append({
            "x_pe_t": np.ascontiguousarray(xs[:B_PE].T),
            "x_dve": np.ascontiguousarray(xs[B_PE:]),
            "w_nat": np.ascontiguousarray(ws[:W_PE]),
            "w_t": np.ascontiguousarray(ws[W_PE:].T),
            "ident": ident,
        })
    return maps


def pe_batch_order(ope_flat: np.ndarray) -> np.ndarray:
    """out_pe [6, 2048] -> batch-ordered [B_PE]. Row r=half*3+slot holds
    4 banks of that half's slot; group e = half*12 + slot*4 + bank."""
    out = np.empty(B_PE, dtype=ope_flat.dtype)
    ope = ope_flat.reshape(6, 4, 512)
    for half in range(2):
        for slot in range(3):
            for bank in range(4):
                e = half * 12 + slot * 4 + bank
                out[e * 512:(e + 1) * 512] = ope[half * 3 + slot, bank]
    return out


def assemble(results) -> np.ndarray:
    out = np.zeros(BATCH, dtype=np.float64)
    for cid in range(N_CORES):
        ope = np.asarray(results[cid]["out_pe"], dtype=np.float64)
        odv = np.asarray(results[cid]["out_dve"], dtype=np.float64)
        out[:B_PE] += pe_batch_order(ope)
        out[B_PE:] += odv.T.reshape(-1)
    return out.astype(np.float32)


def kernel(x: np.ndarray, weight: np.ndarray) -> np.ndarray:
    x = np.asarray(x, dtype=np.float32)
    weight = np.asarray(weight, dtype=np.float32)
    assert x.shape == (BATCH, IN_SIZE) and weight.shape == (W_ROWS, IN_SIZE)
    nc = _get_nc()
    res = run_bass_kernel_spmd(nc, make_in_maps(x, weight), list(range(N_CORES))).results
    return assemble(res)
